# revision 12
# baseline (speedup 1.0000x reference)
"""Trainium2 Bass kernel for nn_AttentionHead_80436147520097.

Single attention head, B=4 T=4096 D=1024 H=64:
    k,q,v = x@W+b;  S[t,s] = k_t . q_s / 8 (causal s<=t);  out = softmax_s(S) @ v

Sharding: 8 cores = 4 batches x 2 parity groups. Within a batch, the two
cores split the softmax (s) dimension by 128-row block parity: core p owns
s-blocks with (block_idx % 2 == p). Every t-chunk's causal extent is a
multiple of 4 blocks, so both parities get exactly half of every chunk's
work -> perfectly balanced AND structurally identical programs (true SPMD,
one NEFF). Odd-parity divergence is pushed into host-prepared data:
  - x rows are 128-block pair-swapped for p==1, so "even device blocks"
    are always the core's own s-blocks (device t order is permuted within
    512-aligned chunks; host un-permutes the output rows).
  - diagonal causal mask tiles are host-computed per parity.
Each core emits partial unnormalized out [T, 65] (col 64 = softmax
denominator) over its s-half; host adds the two halves and divides.

On-chip dataflow (bf16 matmul operands, fp32 PSUM accumulation):
  host sends x transposed+bf16 (xT [D,T]) -> per-512-chunk DMA loads ->
  x-stationary projections: per 128-t-block, stream [Wk|Wq|Wv] (192 cols
  for own s-blocks, 64 k-only cols for the partner parity's blocks)
  through the PE with xT chunks stationary -> psum [128t, 192] -> DVE
  bias-add (bias broadcast rows, free-dim) + cast to bf16 staging ->
  PE transposes k and q blocks (via identity) to kT [64, T] / qT
  [64, 2048] (Pool copies psum->SBUF); v stays natural in v_nat slots
  [128 s, 65] (ones col for the denominator).
  Attention per 512-t-chunk j: S^T strips [128 s-task, 2x512 t] =
  qT-block.T @ kT-chunk per task -> ACT exp (scale 1/8) -> bf16 e ->
  (diag pair: mask multiply) -> PV^T: po[128 t, 65] += e-block.T @
  v_nat-task, accumulated start=False over all tasks in one PSUM bank
  (memset-zeroed; start=True would wipe the 2KB bank row of sibling
  accumulators) -> outbuf [128, 32*65] -> DMA out [T, 65] f32.
"""

import sys

import numpy as np

try:
    import ml_dtypes
except ImportError:  # pragma: no cover
    sys.path.insert(0, "/opt/trn_rl_repo")
    import ml_dtypes

B, T, D, H = 4, 4096, 1024, 64
NCORES = 8
NCHUNK = 8          # t-chunks of 512 per core
NTASK = 16          # own s-tasks (128 rows) per core = T/128/2
BF16 = ml_dtypes.bfloat16

_cache = {}


def _build_program():
    import concourse.bacc as bacc
    import concourse.mybir as mybir
    import concourse.tile as tile
    from concourse import masks as masks_mod

    f32 = mybir.dt.float32
    bf16 = mybir.dt.bfloat16

    nc = bacc.Bacc("TRN2", target_bir_lowering=False, debug=False,
                   num_devices=NCORES)

    xT_d = nc.dram_tensor("xT", [D, T], bf16, kind="ExternalInput").ap()
    wall_d = nc.dram_tensor("wall", [D, 192], bf16, kind="ExternalInput").ap()
    biasb_d = nc.dram_tensor("biasb", [128, 192], f32,
                             kind="ExternalInput").ap()
    mask_d = nc.dram_tensor("mask", [2, 128, 512], bf16,
                            kind="ExternalInput").ap()
    out_d = nc.dram_tensor("out", [T, 65], f32, kind="ExternalOutput").ap()

    with tile.TileContext(nc) as tc:
        with (
            tc.tile_pool(name="const", bufs=1) as const,
            tc.tile_pool(name="xT", bufs=1) as xT_p,
            tc.tile_pool(name="sb", bufs=1) as sb,
            tc.tile_pool(name="stg", bufs=6) as stg_p,
            tc.tile_pool(name="exp", bufs=6) as exp_p,
            tc.tile_pool(name="prps", bufs=2, space="PSUM") as prps,
            tc.tile_pool(name="spair", bufs=2, space="PSUM") as spair_ps,
            tc.tile_pool(name="po", bufs=2, space="PSUM") as out_ps,
        ):
            # ---- constants; wall first so the PE can start ASAP ----
            wall = const.tile([128, 8 * 192], bf16)
            wall3 = wall[:].rearrange("p (c m) -> p c m", c=8)
            nc.sync.dma_start(wall3, wall_d.rearrange("(c p) m -> p c m",
                                                      p=128))
            xT = xT_p.tile([128, 8 * T], bf16)  # col c*T+s = x[s, c*128+p]
            xT3 = xT[:].rearrange("p (c s) -> p c s", c=8)
            xTd3 = xT_d.rearrange("(c p) s -> p c s", p=128)
            for c in range(8):
                nc.sync.dma_start(xT3[:, c, 0:512], xTd3[:, c, 0:512])
            biasb = const.tile([128, 192], f32)
            nc.sync.dma_start(biasb[:], biasb_d)
            masks = const.tile([128, 2 * 512], bf16)
            nc.sync.dma_start(
                masks[:].rearrange("p (m t) -> p m t", m=2),
                mask_d.rearrange("m p t -> p m t"))
            ident = const.tile([128, 128], bf16)
            masks_mod.make_identity(nc, ident[:])
            for u in range(1, NCHUNK):
                for c in range(8):
                    nc.sync.dma_start(xT3[:, c, 512 * u:512 * (u + 1)],
                                      xTd3[:, c, 512 * u:512 * (u + 1)])

            kT = sb.tile([64, T], bf16)
            qT = sb.tile([64, NTASK * 128], bf16)
            v_nat = sb.tile([128, NTASK * 80], bf16)
            ones_col = v_nat[:].rearrange("p (n w) -> p n w", w=80)[:, :, 64:65]
            nc.vector.memset(ones_col, 1.0)
            outbuf = sb.tile([128, 32 * 65], f32)
            # prefetch ACT exp table set off the critical path
            scratch = const.tile([1, 8], f32)
            nc.vector.memset(scratch[:], 0.0)
            nc.scalar.activation(scratch[:], scratch[:],
                                 mybir.ActivationFunctionType.Exp)

            # ---- x-stationary projections for one 512-t-chunk ----
            def proj_mms(u):
                stgs = []
                for i in range(4):
                    db = 4 * u + i
                    w = 192 if i % 2 == 0 else 64  # own blocks: k|q|v
                    ps = prps.tile([128, 192], f32, tag="prps")
                    for c in range(8):
                        nc.tensor.matmul(
                            ps[:, 0:w],
                            xT3[:, c, db * 128:(db + 1) * 128],
                            wall3[:, c, 0:w],
                            start=(c == 0), stop=(c == 7),
                            skip_group_check=True,
                        )
                    stg = stg_p.tile([128, 192], bf16, tag="stg")
                    nc.vector.tensor_add(stg[:, 0:w], ps[:, 0:w],
                                         biasb[:, 0:w])
                    stgs.append(stg)
                    if i % 2 == 0:  # v natural for own task (Pool: SBUF-only)
                        ts = 2 * u + i // 2
                        nc.gpsimd.tensor_copy(
                            v_nat[:, ts * 80: ts * 80 + 64],
                            stg[:, 128:192])
                return stgs

            def make_transposes(u, stgs):
                outs = []

                def emit(stg, col0, dst, dcol):
                    def run():
                        # share the prps slot carousel (same space+bytes:
                        # [128,384] bf16 == [128,192] f32 == 768 B/part)
                        pt = prps.tile([128, 384], bf16, tag="prps")
                        nc.tensor.transpose(pt[0:64, 0:128],
                                            stg[:, col0:col0 + 64],
                                            ident[:])
                        nc.vector.tensor_copy(dst[:, dcol:dcol + 128],
                                              pt[0:64, 0:128])
                    return run

                for i in range(4):
                    db = 4 * u + i
                    outs.append(emit(stgs[i], 0, kT, db * 128))
                for i in (0, 2):
                    ts = 2 * u + i // 2
                    outs.append(emit(stgs[i], 64, qT, ts * 128))
                return outs

            # ---- attention for one 512-t-chunk ----
            def attn_chunk(j, inject):
                po = out_ps.tile([128, 4 * 65], f32, tag="po")
                nc.vector.memset(po[:], 0.0)
                kcol = j * 512
                for tp in range(j + 1):
                    ps = spair_ps.tile([128, 1024], f32, tag="spair")
                    e = exp_p.tile([128, 1024], bf16, tag="exp")
                    for h in range(2):
                        ts = 2 * tp + h
                        nc.tensor.matmul(
                            ps[:, h * 512:(h + 1) * 512],
                            qT[:, ts * 128:(ts + 1) * 128],
                            kT[:, kcol:kcol + 512],
                            start=True, stop=True, skip_group_check=True,
                        )
                        if inject:
                            inject.pop(0)()
                    nc.scalar.activation(
                        e[:], ps[:], mybir.ActivationFunctionType.Exp,
                        scale=0.125)
                    if tp == j:  # diagonal pair: causal mask (Pool: SBUF-only)
                        nc.gpsimd.tensor_mul(
                            e[:, 0:512], e[:, 0:512], masks[:, 0:512])
                        nc.gpsimd.tensor_mul(
                            e[:, 512:1024], e[:, 512:1024], masks[:, 512:1024])
                    for h in range(2):
                        ts = 2 * tp + h
                        for tb in range(4):
                            nc.tensor.matmul(
                                po[:, tb * 65:(tb + 1) * 65],
                                e[:, h * 512 + tb * 128:
                                   h * 512 + (tb + 1) * 128],
                                v_nat[:, ts * 80: ts * 80 + 65],
                                start=False,
                                stop=(tp == j and h == 1),
                                skip_group_check=True,
                            )
                while inject:
                    inject.pop(0)()
                nc.vector.tensor_copy(
                    outbuf[:, j * 260:(j + 1) * 260], po[:])

            stgs = proj_mms(0)
            tr_prev = make_transposes(0, stgs)
            for t in tr_prev:
                t()
            for u in range(1, NCHUNK):
                stgs = proj_mms(u)
                attn_chunk(u - 1, make_transposes(u, stgs))
            attn_chunk(NCHUNK - 1, [])

            outd3 = out_d.rearrange("(blk p) h -> p blk h", p=128)
            outb3 = outbuf[:].rearrange("p (blk h) -> p blk h", h=65)
            for j in range(NCHUNK):
                nc.sync.dma_start(outd3[:, 4 * j:4 * (j + 1), :],
                                  outb3[:, 4 * j:4 * (j + 1), :])
    nc.compile()
    return nc


def _host_masks():
    """mask[parity][m][s, t'] over device-t coords within a 512 chunk."""
    out = np.zeros((2, 2, 128, 512), dtype=BF16)
    s = np.arange(128)[:, None]
    tp = np.arange(512)[None, :]
    for p in range(2):
        if p == 0:
            t_orig = tp
        else:  # device blocks pair-swapped
            t_orig = ((tp // 128) ^ 1) * 128 + tp % 128
        for m in range(2):
            s_orig = (2 * m + p) * 128 + s
            out[p, m] = (s_orig <= t_orig).astype(BF16)
    return out


def kernel(x, Wk, bk, Wq, bq, Wv, bv):
    from concourse.bass_utils import run_bass_kernel_spmd

    if "nc" not in _cache:
        _cache["nc"] = _build_program()
    nc = _cache["nc"]

    x = np.asarray(x, np.float32)
    wall = np.concatenate(
        [np.asarray(Wk), np.asarray(Wq), np.asarray(Wv)], axis=1).astype(BF16)
    biasb = np.tile(
        np.concatenate([np.asarray(bk), np.asarray(bq), np.asarray(bv)])
        .astype(np.float32)[None, :], (128, 1))
    masks = _host_masks()

    in_maps = []
    for core in range(NCORES):
        b, p = core // 2, core % 2
        xb = x[b]
        if p == 1:  # swap 128-row blocks within 256-row pairs
            xb = xb.reshape(T // 256, 2, 128, D)[:, ::-1].reshape(T, D)
        in_maps.append({
            "xT": np.ascontiguousarray(xb.T.astype(BF16)),
            "wall": wall,
            "biasb": biasb,
            "mask": np.ascontiguousarray(masks[p]),
        })

    res = run_bass_kernel_spmd(nc, in_maps, core_ids=list(range(NCORES)))
    results = res.results
    _cache["last_run"] = res

    out = np.zeros((B, T, H), np.float32)
    for b in range(B):
        a0 = results[2 * b]["out"]        # [T, 65] device-t natural
        a1 = results[2 * b + 1]["out"]    # [T, 65] device-t pair-swapped
        a1 = a1.reshape(T // 256, 2, 128, 65)[:, ::-1].reshape(T, 65)
        tot = a0 + a1
        out[b] = tot[:, 0:64] / tot[:, 64:65]
    return out
